# revision 29
# baseline (speedup 1.0000x reference)
"""Trainium2 Bass kernel for pointer-generator style attention.

reference math (per batch b):
    dec_fea = s_t_hat @ W_dec + b_dec                    # (B, H)
    e       = tanh(encoder_feature.reshape(B,T,H) + dec_fea[:,None,:])
    scores  = einsum('bth,h->bt', e, v_w)
    attn    = softmax(scores, axis=1)  (then renorm - a no-op)
    c_t     = einsum('bt,bth->bh', attn, encoder_outputs)
    returns (c_t, attn, coverage)       # coverage passes through

Sharding: data-parallel over batch. 8 cores x 4 batches each. Weights
replicated. No collectives.

Per-core pipeline (memory-bound: 64MB of encoder tensors / core):
  - ef/eo stream in 1MB tiles [128, 4*512] (t = j*512 + 4p + k).
    ef on the Sync HWDGE queue, eo on the Scalar HWDGE queue, setup +
    outputs on the GpSimd (SWDGE) queue - three queues stream in parallel.
  - z = ef + dec_bcast          one DVE tensor_tensor per 1MB tile
  - e = tanh(z)                 one ACT op per 1MB tile
  - score_col = sum_h(e * v_bcast)  fused DVE AFFINE_MUL_REDUCE per 512-col
  - softmax over t: DVE free-dim reduce + GPSIMD partition_all_reduce,
    ACT exp with bias=-max and accum_out row sums.
  - c_t: PE fp32r matmuls attn_col.T @ eo_sub accumulated in PSUM.
  - attn written back with a strided scatter DMA (64KB total per core).
"""

import numpy as np
from contextlib import ExitStack

import concourse.bass as bass
import concourse.tile as tile
from concourse import mybir, bass_isa, bacc
from concourse.bass_utils import run_bass_kernel_spmd

B, T, H, NIN = 32, 4096, 512, 1024
NCORES = 8
BL = B // NCORES          # batches per core
PT = 128                  # partitions
KT = 4                    # t-rows per partition within one 1MB tile
F32 = mybir.dt.float32
F32R = mybir.dt.float32r


def build(bl=BL, t=T, h=H, nin=NIN):
    """Build the single-core Bass program (same program runs SPMD on all cores)."""
    nj = t // (PT * KT)    # 1MB tiles per batch (8 at full size)
    ncols = nj * KT        # score columns per batch (32)
    nc = bacc.Bacc("TRN2", target_bir_lowering=False, debug=False)

    ef = nc.dram_tensor("ef", [bl * t, h], F32, kind="ExternalInput").ap()
    eo = nc.dram_tensor("eo", [bl, t, h], F32, kind="ExternalInput").ap()
    stT = nc.dram_tensor("stT", [nin, bl], F32, kind="ExternalInput").ap()
    wdec = nc.dram_tensor("wdec", [nin, h], F32, kind="ExternalInput").ap()
    bdec = nc.dram_tensor("bdec", [1, h], F32, kind="ExternalInput").ap()
    vw = nc.dram_tensor("vw", [1, h], F32, kind="ExternalInput").ap()
    ct_out = nc.dram_tensor("ct", [bl, h], F32, kind="ExternalOutput").ap()
    attn_out = nc.dram_tensor("attn", [bl, t], F32, kind="ExternalOutput").ap()

    # tile (b, j) is 1MB contiguous; partition p holds t-rows 4p..4p+3
    ef_v = ef.rearrange("(b j p k) h -> b j p (k h)", b=bl, j=nj, p=PT, k=KT)
    eo_v = eo.rearrange("b (j p k) h -> b j p (k h)", j=nj, p=PT, k=KT)

    def attn_dram_ap(b):
        # [p][j][k] nest: addr = b*t + j*(PT*KT) + p*KT + k
        return bass.AP(
            tensor=attn_out.tensor,
            offset=attn_out.offset + b * t,
            ap=[[KT, PT], [PT * KT, nj], [1, KT]],
        )

    with tile.TileContext(nc) as tc, ExitStack() as ctx:
        singles = ctx.enter_context(tc.tile_pool(name="singles", bufs=1))
        dram_pool = ctx.enter_context(tc.tile_pool(name="dram", bufs=1, space="DRAM"))
        ef_pool = ctx.enter_context(tc.tile_pool(name="efp", bufs=3))
        eo_pool = ctx.enter_context(tc.tile_pool(name="eop", bufs=nj + 2))
        z_pool = ctx.enter_context(tc.tile_pool(name="zp", bufs=2))
        e_pool = ctx.enter_context(tc.tile_pool(name="ep", bufs=3))
        scores_pool = ctx.enter_context(tc.tile_pool(name="scoresp", bufs=2))
        attn_pool = ctx.enter_context(tc.tile_pool(name="attnp", bufs=2))
        small = ctx.enter_context(tc.tile_pool(name="small", bufs=2))
        ps_ct = ctx.enter_context(tc.tile_pool(name="psct", bufs=2, space="PSUM"))
        ps_dec = ctx.enter_context(tc.tile_pool(name="psdec", bufs=1, space="PSUM"))

        nk = nin // PT
        # ---- constants / small inputs (all on the GpSimd SWDGE queue)
        w_sb = singles.tile([PT, nk, h], F32)
        nc.gpsimd.dma_start(w_sb, wdec.rearrange("(c p) h -> p c h", p=PT))
        st_sb = singles.tile([PT, nk, bl], F32)
        nc.gpsimd.dma_start(st_sb, stT.rearrange("(c p) b -> p c b", p=PT))
        bdec_sb = singles.tile([1, h], F32)
        nc.gpsimd.dma_start(bdec_sb, bdec)
        ones1 = singles.tile([1, bl], F32)
        nc.vector.memset(ones1, 1.0)
        # v broadcast to all 128 partitions via stride-0 DMA from DRAM
        v_bc = singles.tile([PT, h], F32)
        nc.gpsimd.dma_start(
            v_bc, bass.AP(tensor=vw.tensor, offset=vw.offset, ap=[[0, PT], [1, h]])
        )

        # ---- dec_fea = s_t_hat @ W_dec + b_dec  (exact fp32) -> [bl, h]
        dec_ps = ps_dec.tile([bl, h], F32)
        for c in range(nk):
            nc.tensor.matmul(
                dec_ps, lhsT=st_sb[:, c, :], rhs=w_sb[:, c, :],
                start=(c == 0), stop=False,
            )
        nc.tensor.matmul(dec_ps, lhsT=ones1, rhs=bdec_sb, start=False, stop=True)
        dec_sb = singles.tile([bl, h], F32)
        nc.vector.tensor_copy(dec_sb, dec_ps)
        # broadcast each batch row to all partitions via DRAM round-trip
        dec_dram = dram_pool.tile([bl, h], F32)
        nc.gpsimd.dma_start(dec_dram, dec_sb)
        dec_bc = singles.tile([PT, bl, h], F32)
        for b in range(bl):
            nc.gpsimd.dma_start(
                dec_bc[:, b, :],
                bass.AP(tensor=dec_dram.tensor, offset=dec_dram.offset + b * h,
                        ap=[[0, PT], [1, h]]),
            )

        # ---- main loop
        for b in range(bl):
            scores_t = scores_pool.tile([PT, ncols], F32)
            for j in range(nj):
                ef_t = ef_pool.tile([PT, KT, h], F32)
                nc.sync.dma_start(ef_t, ef_v[b, j])
                z_t = z_pool.tile([PT, KT * h], F32)
                dslice = dec_bc[:, b, :]
                dec_rep = bass.AP(
                    tensor=dslice.tensor, offset=dslice.offset,
                    ap=[list(dslice.ap[0]), [0, KT], list(dslice.ap[1])],
                )
                # DVE is the co-bottleneck engine: push 2 of 8 adds per batch
                # to the otherwise-idle GPSIMD engine
                add_eng = nc.gpsimd if j % 4 == 3 else nc.vector
                add_eng.tensor_add(
                    z_t.rearrange("p (k h) -> p k h", k=KT),
                    ef_t, dec_rep,
                )
                e_t = e_pool.tile([PT, KT, h], F32)
                nc.scalar.activation(
                    e_t.rearrange("p k h -> p (k h)"), z_t,
                    mybir.ActivationFunctionType.Tanh,
                )
                for k in range(KT):
                    dummy = small.tile([PT, 1], F32)
                    nc.vector.affine_mul_reduce(
                        dummy.broadcast_to([PT, h]),
                        scores_t[:, j * KT + k : j * KT + k + 1],
                        e_t[:, k, :], v_bc, 1.0, 0.0,
                    )

            # ---- softmax over t. No max-subtraction: |scores| <= sum|v_w|
            # (~8), exp cannot overflow in fp32, and softmax is shift-exact.
            ex = attn_pool.tile([PT, ncols], F32)
            rowsum = small.tile([PT, 1], F32)
            nc.scalar.activation(
                ex, scores_t, mybir.ActivationFunctionType.Exp,
                accum_out=rowsum,
            )
            # unnormalized weights feed the PE right away; 1/Z is applied to
            # c_t afterwards, so the all-reduce is off the critical path
            ex_r = attn_pool.tile([PT, ncols], F32R)
            nc.vector.tensor_copy(ex_r, ex)
            zsum = small.tile([PT, 1], F32)
            nc.gpsimd.partition_all_reduce(zsum, rowsum, PT, bass_isa.ReduceOp.add)
            rz = small.tile([PT, 1], F32)
            nc.vector.reciprocal(rz, zsum)
            at = attn_pool.tile([PT, ncols], F32)
            nc.vector.tensor_scalar_mul(at, ex, rz)

            # ---- attn output: strided scatter DMA (16KB per batch)
            nc.gpsimd.dma_start(attn_dram_ap(b), at)

            # ---- c_t = exp @ encoder_outputs * (1/Z)  (fp32r PE)
            # eo loads issued lazily here (Scalar HWDGE queue): issuing them
            # earlier steals HBM bandwidth from the latency-critical ef stream
            ct_ps = ps_ct.tile([1, h], F32)
            for j in range(nj):
                eo_t = eo_pool.tile([PT, KT, h], F32R)
                nc.scalar.dma_start(eo_t, eo_v[b, j].bitcast(F32R))
                for k in range(KT):
                    jk = j * KT + k
                    nc.tensor.matmul(
                        ct_ps, lhsT=ex_r[:, jk : jk + 1], rhs=eo_t[:, k, :],
                        start=(jk == 0), stop=(jk == nj * KT - 1),
                    )
            ct_sb = small.tile([1, h], F32)
            nc.vector.tensor_scalar_mul(ct_sb, ct_ps, rz[0:1, :])
            nc.gpsimd.dma_start(ct_out[b : b + 1, :], ct_sb)

    nc.compile()
    return nc


_NC_CACHE = {}


def _get_nc():
    if "nc" not in _NC_CACHE:
        _NC_CACHE["nc"] = build()
    return _NC_CACHE["nc"]


def make_in_maps(s_t_hat, encoder_outputs, encoder_feature, W_dec, b_dec, v_w,
                 bl=BL, ncores=NCORES):
    s_t_hat = np.asarray(s_t_hat, dtype=np.float32)
    encoder_outputs = np.asarray(encoder_outputs, dtype=np.float32)
    encoder_feature = np.asarray(encoder_feature, dtype=np.float32)
    W_dec = np.ascontiguousarray(np.asarray(W_dec, dtype=np.float32))
    b_dec = np.ascontiguousarray(np.asarray(b_dec, dtype=np.float32).reshape(1, -1))
    v_w = np.ascontiguousarray(np.asarray(v_w, dtype=np.float32).reshape(1, -1))
    b = s_t_hat.shape[0]
    t = encoder_outputs.shape[1]
    hh = encoder_outputs.shape[2]
    ef3 = encoder_feature.reshape(b, t, hh)
    in_maps = []
    for i in range(ncores):
        sl = slice(i * bl, (i + 1) * bl)
        in_maps.append({
            "ef": np.ascontiguousarray(ef3[sl].reshape(bl * t, hh)),
            "eo": np.ascontiguousarray(encoder_outputs[sl]),
            "stT": np.ascontiguousarray(s_t_hat[sl].T),
            "wdec": W_dec,
            "bdec": b_dec,
            "vw": v_w,
        })
    return in_maps


def kernel(s_t_hat, encoder_outputs, encoder_feature, coverage, W_dec, b_dec, v_w):
    coverage = np.asarray(coverage, dtype=np.float32)
    nc = _get_nc()
    in_maps = make_in_maps(
        s_t_hat, encoder_outputs, encoder_feature, W_dec, b_dec, v_w
    )
    res = None
    last_err = None
    for attempt in range(3):
        try:
            res = run_bass_kernel_spmd(nc, in_maps, list(range(NCORES))).results
            break
        except Exception as ex:  # transient NRT_EXEC_UNIT_UNRECOVERABLE flake
            last_err = ex
            try:
                import jax
                jax.clear_caches()
                jax.extend.backend.clear_backends()
            except Exception:
                pass
    if res is None:
        raise last_err
    c_t = np.concatenate([res[i]["ct"] for i in range(NCORES)], axis=0)
    attn = np.concatenate([res[i]["attn"] for i in range(NCORES)], axis=0)
    return (c_t, attn, coverage)


# revision 35
# speedup vs baseline: 1.1453x; 1.1453x over previous
"""Trainium2 Bass kernel for pointer-generator style attention.

reference math (per batch b):
    dec_fea = s_t_hat @ W_dec + b_dec                    # (B, H)
    e       = tanh(encoder_feature.reshape(B,T,H) + dec_fea[:,None,:])
    scores  = einsum('bth,h->bt', e, v_w)
    attn    = softmax(scores, axis=1)  (then renorm - a no-op)
    c_t     = einsum('bt,bth->bh', attn, encoder_outputs)
    returns (c_t, attn, coverage)       # coverage passes through

Sharding: data-parallel over batch. 8 cores x 4 batches each. Weights
replicated. No collectives.

Per-core pipeline (memory-bound: 64MB of encoder tensors / core):
  - ef/eo stream in 1MB tiles [128, 4*512] (t = j*512 + 4p + k).
    ef on the Sync HWDGE queue, eo on the Scalar HWDGE queue, setup +
    outputs on the GpSimd (SWDGE) queue - three queues stream in parallel.
  - z = ef + dec_bcast          one DVE tensor_tensor per 1MB tile
  - e = tanh(z)                 one ACT op per 1MB tile
  - score_col = sum_h(e * v_bcast)  fused DVE AFFINE_MUL_REDUCE per 512-col
  - softmax over t: DVE free-dim reduce + GPSIMD partition_all_reduce,
    ACT exp with bias=-max and accum_out row sums.
  - c_t: PE fp32r matmuls attn_col.T @ eo_sub accumulated in PSUM.
  - attn written back with a strided scatter DMA (64KB total per core).
"""

import numpy as np
from contextlib import ExitStack

import concourse.bass as bass
import concourse.tile as tile
from concourse import mybir, bass_isa, bacc
from concourse.bass_utils import run_bass_kernel_spmd

B, T, H, NIN = 32, 4096, 512, 1024
NCORES = 8
BL = B // NCORES          # batches per core
PT = 128                  # partitions
KT = 4                    # t-rows per partition within one 1MB tile
F32 = mybir.dt.float32
F32R = mybir.dt.float32r


def build(bl=BL, t=T, h=H, nin=NIN):
    """Build the single-core Bass program (same program runs SPMD on all cores)."""
    nj = t // (PT * KT)    # 1MB tiles per batch (8 at full size)
    ncols = nj * KT        # score columns per batch (32)
    nc = bacc.Bacc("TRN2", target_bir_lowering=False, debug=False)

    ef = nc.dram_tensor("ef", [bl * t, h], F32, kind="ExternalInput").ap()
    eo = nc.dram_tensor("eo", [bl, t, h], F32, kind="ExternalInput").ap()
    stT = nc.dram_tensor("stT", [nin, bl], F32, kind="ExternalInput").ap()
    wdec = nc.dram_tensor("wdec", [nin, h], F32, kind="ExternalInput").ap()
    bdec = nc.dram_tensor("bdec", [1, h], F32, kind="ExternalInput").ap()
    vw = nc.dram_tensor("vw", [1, h], F32, kind="ExternalInput").ap()
    ct_out = nc.dram_tensor("ct", [bl, h], F32, kind="ExternalOutput").ap()
    attn_out = nc.dram_tensor("attn", [bl, t], F32, kind="ExternalOutput").ap()

    # tile (b, j) is 1MB contiguous; partition p holds t-rows 4p..4p+3
    ef_v = ef.rearrange("(b j p k) h -> b j p (k h)", b=bl, j=nj, p=PT, k=KT)
    eo_v = eo.rearrange("b (j p k) h -> b j p (k h)", j=nj, p=PT, k=KT)

    def attn_dram_ap(b):
        # [p][j][k] nest: addr = b*t + j*(PT*KT) + p*KT + k
        return bass.AP(
            tensor=attn_out.tensor,
            offset=attn_out.offset + b * t,
            ap=[[KT, PT], [PT * KT, nj], [1, KT]],
        )

    with tile.TileContext(nc) as tc, ExitStack() as ctx:
        singles = ctx.enter_context(tc.tile_pool(name="singles", bufs=1))
        dram_pool = ctx.enter_context(tc.tile_pool(name="dram", bufs=1, space="DRAM"))
        ef_pool = ctx.enter_context(tc.tile_pool(name="efp", bufs=4))
        eo_pool = ctx.enter_context(tc.tile_pool(name="eop", bufs=nj + 2))
        z_pool = ctx.enter_context(tc.tile_pool(name="zp", bufs=2))
        e_pool = ctx.enter_context(tc.tile_pool(name="ep", bufs=3))
        scores_pool = ctx.enter_context(tc.tile_pool(name="scoresp", bufs=2))
        attn_pool = ctx.enter_context(tc.tile_pool(name="attnp", bufs=2))
        small = ctx.enter_context(tc.tile_pool(name="small", bufs=2))
        ps_ct = ctx.enter_context(tc.tile_pool(name="psct", bufs=2, space="PSUM"))
        ps_dec = ctx.enter_context(tc.tile_pool(name="psdec", bufs=1, space="PSUM"))
        ps_warm = ctx.enter_context(tc.tile_pool(name="pswarm", bufs=1, space="PSUM"))

        nk = nin // PT
        # ---- constants / small inputs (all on the GpSimd SWDGE queue)
        w_sb = singles.tile([PT, nk, h], F32)
        nc.gpsimd.dma_start(w_sb, wdec.rearrange("(c p) h -> p c h", p=PT))
        st_sb = singles.tile([PT, nk, bl], F32)
        nc.gpsimd.dma_start(st_sb, stT.rearrange("(c p) b -> p c b", p=PT))
        bdec_sb = singles.tile([1, h], F32)
        nc.gpsimd.dma_start(bdec_sb, bdec)
        ones1 = singles.tile([1, bl], F32)
        nc.vector.memset(ones1, 1.0)
        # v broadcast to all 128 partitions via stride-0 DMA from DRAM
        v_bc = singles.tile([PT, h], F32)
        nc.gpsimd.dma_start(
            v_bc, bass.AP(tensor=vw.tensor, offset=vw.offset, ap=[[0, PT], [1, h]])
        )

        # ---- dec_fea = s_t_hat @ W_dec + b_dec  (exact fp32) -> [bl, h]
        dec_ps = ps_dec.tile([bl, h], F32)
        for c in range(nk):
            nc.tensor.matmul(
                dec_ps, lhsT=st_sb[:, c, :], rhs=w_sb[:, c, :],
                start=(c == 0), stop=False,
            )
        nc.tensor.matmul(dec_ps, lhsT=ones1, rhs=bdec_sb, start=False, stop=True)
        dec_sb = singles.tile([bl, h], F32)
        nc.vector.tensor_copy(dec_sb, dec_ps)
        # broadcast each batch row to all partitions via DRAM round-trip
        dec_dram = dram_pool.tile([bl, h], F32)
        nc.gpsimd.dma_start(dec_dram, dec_sb)
        dec_bc = singles.tile([PT, bl, h], F32)
        for b in range(bl):
            nc.gpsimd.dma_start(
                dec_bc[:, b, :],
                bass.AP(tensor=dec_dram.tensor, offset=dec_dram.offset + b * h,
                        ap=[[0, PT], [1, h]]),
            )

        # ---- main loop
        for b in range(bl):
            scores_t = scores_pool.tile([PT, ncols], F32)
            last_e = []
            for j in range(nj):
                ef_t = ef_pool.tile([PT, KT, h], F32)
                nc.sync.dma_start(ef_t, ef_v[b, j])
                z_t = z_pool.tile([PT, KT * h], F32)
                dslice = dec_bc[:, b, :]
                dec_rep = bass.AP(
                    tensor=dslice.tensor, offset=dslice.offset,
                    ap=[list(dslice.ap[0]), [0, KT], list(dslice.ap[1])],
                )
                nc.vector.tensor_add(
                    z_t.rearrange("p (k h) -> p k h", k=KT),
                    ef_t, dec_rep,
                )
                e_t = e_pool.tile([PT, KT, h], F32)
                nc.scalar.activation(
                    e_t.rearrange("p k h -> p (k h)"), z_t,
                    mybir.ActivationFunctionType.Tanh,
                )
                for k in range(KT):
                    dummy = small.tile([PT, 1], F32)
                    nc.vector.affine_mul_reduce(
                        dummy.broadcast_to([PT, h]),
                        scores_t[:, j * KT + k : j * KT + k + 1],
                        e_t[:, k, :], v_bc, 1.0, 0.0,
                    )
                if b == bl - 1 and j >= nj - 2:
                    last_e.append(e_t)

            if b == bl - 1:
                # Warm the PE's HAM clock gate right before the final ct burst
                # (otherwise it idles through the scores phase and the last
                # batch's 32 matmuls run at the cold 1.2GHz rate, fully
                # exposed in the kernel tail). Chained off the last e-tiles
                # so these execute late; sunk to DRAM so DCE keeps them.
                warm = ps_warm.tile([PT, h], F32)
                for w in range(6):
                    src = last_e[w % len(last_e)]
                    nc.tensor.matmul(
                        warm, lhsT=src[:, w % KT, 0:PT], rhs=src[:, w % KT, :],
                        start=(w == 0), stop=(w == 5),
                    )
                sink = small.tile([1, 1], F32)
                nc.vector.tensor_copy(sink, warm[0:1, 0:1])
                warm_dram = dram_pool.tile([1, 1], F32)
                nc.gpsimd.dma_start(warm_dram, sink)

            # ---- softmax over t. No max-subtraction: |scores| <= sum|v_w|
            # (~8), exp cannot overflow in fp32, and softmax is shift-exact.
            ex = attn_pool.tile([PT, ncols], F32)
            rowsum = small.tile([PT, 1], F32)
            nc.scalar.activation(
                ex, scores_t, mybir.ActivationFunctionType.Exp,
                accum_out=rowsum,
            )
            # unnormalized weights feed the PE right away; 1/Z is applied to
            # c_t afterwards, so the all-reduce is off the critical path
            ex_r = attn_pool.tile([PT, ncols], F32R)
            nc.vector.tensor_copy(ex_r, ex)
            zsum = small.tile([PT, 1], F32)
            nc.gpsimd.partition_all_reduce(zsum, rowsum, PT, bass_isa.ReduceOp.add)
            rz = small.tile([PT, 1], F32)
            nc.vector.reciprocal(rz, zsum)
            at = attn_pool.tile([PT, ncols], F32)
            nc.vector.tensor_scalar_mul(at, ex, rz)

            # ---- attn output: strided scatter DMA (16KB per batch)
            nc.gpsimd.dma_start(attn_dram_ap(b), at)

            # ---- c_t = exp @ encoder_outputs * (1/Z)  (fp32r PE)
            # eo loads issued lazily here (Scalar HWDGE queue): issuing them
            # earlier steals HBM bandwidth from the latency-critical ef stream
            ct_ps = ps_ct.tile([1, h], F32)
            for j in range(nj):
                eo_t = eo_pool.tile([PT, KT, h], F32R)
                nc.scalar.dma_start(eo_t, eo_v[b, j].bitcast(F32R))
                for k in range(KT):
                    jk = j * KT + k
                    nc.tensor.matmul(
                        ct_ps, lhsT=ex_r[:, jk : jk + 1], rhs=eo_t[:, k, :],
                        start=(jk == 0), stop=(jk == nj * KT - 1),
                    )
            ct_sb = small.tile([1, h], F32)
            nc.vector.tensor_scalar_mul(ct_sb, ct_ps, rz[0:1, :])
            nc.gpsimd.dma_start(ct_out[b : b + 1, :], ct_sb)

    nc.compile()
    return nc


_NC_CACHE = {}


def _get_nc():
    if "nc" not in _NC_CACHE:
        _NC_CACHE["nc"] = build()
    return _NC_CACHE["nc"]


def make_in_maps(s_t_hat, encoder_outputs, encoder_feature, W_dec, b_dec, v_w,
                 bl=BL, ncores=NCORES):
    s_t_hat = np.asarray(s_t_hat, dtype=np.float32)
    encoder_outputs = np.asarray(encoder_outputs, dtype=np.float32)
    encoder_feature = np.asarray(encoder_feature, dtype=np.float32)
    W_dec = np.ascontiguousarray(np.asarray(W_dec, dtype=np.float32))
    b_dec = np.ascontiguousarray(np.asarray(b_dec, dtype=np.float32).reshape(1, -1))
    v_w = np.ascontiguousarray(np.asarray(v_w, dtype=np.float32).reshape(1, -1))
    b = s_t_hat.shape[0]
    t = encoder_outputs.shape[1]
    hh = encoder_outputs.shape[2]
    ef3 = encoder_feature.reshape(b, t, hh)
    in_maps = []
    for i in range(ncores):
        sl = slice(i * bl, (i + 1) * bl)
        in_maps.append({
            "ef": np.ascontiguousarray(ef3[sl].reshape(bl * t, hh)),
            "eo": np.ascontiguousarray(encoder_outputs[sl]),
            "stT": np.ascontiguousarray(s_t_hat[sl].T),
            "wdec": W_dec,
            "bdec": b_dec,
            "vw": v_w,
        })
    return in_maps


def kernel(s_t_hat, encoder_outputs, encoder_feature, coverage, W_dec, b_dec, v_w):
    coverage = np.asarray(coverage, dtype=np.float32)
    nc = _get_nc()
    in_maps = make_in_maps(
        s_t_hat, encoder_outputs, encoder_feature, W_dec, b_dec, v_w
    )
    res = None
    last_err = None
    for attempt in range(3):
        try:
            res = run_bass_kernel_spmd(nc, in_maps, list(range(NCORES))).results
            break
        except Exception as ex:  # transient NRT_EXEC_UNIT_UNRECOVERABLE flake
            last_err = ex
            try:
                import jax
                jax.clear_caches()
                jax.extend.backend.clear_backends()
            except Exception:
                pass
    if res is None:
        raise last_err
    c_t = np.concatenate([res[i]["ct"] for i in range(NCORES)], axis=0)
    attn = np.concatenate([res[i]["attn"] for i in range(NCORES)], axis=0)
    return (c_t, attn, coverage)


# revision 36
# speedup vs baseline: 1.2256x; 1.0701x over previous
"""Trainium2 Bass kernel for pointer-generator style attention.

reference math (per batch b):
    dec_fea = s_t_hat @ W_dec + b_dec                    # (B, H)
    e       = tanh(encoder_feature.reshape(B,T,H) + dec_fea[:,None,:])
    scores  = einsum('bth,h->bt', e, v_w)
    attn    = softmax(scores, axis=1)  (then renorm - a no-op)
    c_t     = einsum('bt,bth->bh', attn, encoder_outputs)
    returns (c_t, attn, coverage)       # coverage passes through

Sharding: data-parallel over batch. 8 cores x 4 batches each. Weights
replicated. No collectives.

Per-core pipeline (memory-bound: 64MB of encoder tensors / core):
  - ef/eo stream in 1MB tiles [128, 4*512] (t = j*512 + 4p + k).
    ef on the Sync HWDGE queue, eo on the Scalar HWDGE queue, setup +
    outputs on the GpSimd (SWDGE) queue - three queues stream in parallel.
  - z = ef + dec_bcast          one DVE tensor_tensor per 1MB tile
  - e = tanh(z)                 one ACT op per 1MB tile
  - score_col = sum_h(e * v_bcast)  fused DVE AFFINE_MUL_REDUCE per 512-col
  - softmax over t: DVE free-dim reduce + GPSIMD partition_all_reduce,
    ACT exp with bias=-max and accum_out row sums.
  - c_t: PE fp32r matmuls attn_col.T @ eo_sub accumulated in PSUM.
  - attn written back with a strided scatter DMA (64KB total per core).
"""

import numpy as np
from contextlib import ExitStack

import concourse.bass as bass
import concourse.tile as tile
from concourse import mybir, bass_isa, bacc
from concourse.bass_utils import run_bass_kernel_spmd

B, T, H, NIN = 32, 4096, 512, 1024
NCORES = 8
BL = B // NCORES          # batches per core
PT = 128                  # partitions
KT = 4                    # t-rows per partition within one 1MB tile
F32 = mybir.dt.float32
F32R = mybir.dt.float32r


def build(bl=BL, t=T, h=H, nin=NIN):
    """Build the single-core Bass program (same program runs SPMD on all cores)."""
    nj = t // (PT * KT)    # 1MB tiles per batch (8 at full size)
    ncols = nj * KT        # score columns per batch (32)
    nc = bacc.Bacc("TRN2", target_bir_lowering=False, debug=False)

    ef = nc.dram_tensor("ef", [bl * t, h], F32, kind="ExternalInput").ap()
    eo = nc.dram_tensor("eo", [bl, t, h], F32, kind="ExternalInput").ap()
    stT = nc.dram_tensor("stT", [nin, bl], F32, kind="ExternalInput").ap()
    wdec = nc.dram_tensor("wdec", [nin, h], F32, kind="ExternalInput").ap()
    bdec = nc.dram_tensor("bdec", [1, h], F32, kind="ExternalInput").ap()
    vw = nc.dram_tensor("vw", [1, h], F32, kind="ExternalInput").ap()
    ct_out = nc.dram_tensor("ct", [bl, h], F32, kind="ExternalOutput").ap()
    attn_out = nc.dram_tensor("attn", [bl, t], F32, kind="ExternalOutput").ap()

    # tile (b, j) is 1MB contiguous; partition p holds t-rows 4p..4p+3
    ef_v = ef.rearrange("(b j p k) h -> b j p (k h)", b=bl, j=nj, p=PT, k=KT)
    eo_v = eo.rearrange("b (j p k) h -> b j p (k h)", j=nj, p=PT, k=KT)

    def attn_dram_ap(b):
        # [p][j][k] nest: addr = b*t + j*(PT*KT) + p*KT + k
        return bass.AP(
            tensor=attn_out.tensor,
            offset=attn_out.offset + b * t,
            ap=[[KT, PT], [PT * KT, nj], [1, KT]],
        )

    with tile.TileContext(nc) as tc, ExitStack() as ctx:
        singles = ctx.enter_context(tc.tile_pool(name="singles", bufs=1))
        dram_pool = ctx.enter_context(tc.tile_pool(name="dram", bufs=1, space="DRAM"))
        ef_pool = ctx.enter_context(tc.tile_pool(name="efp", bufs=3))
        eo_pool = ctx.enter_context(tc.tile_pool(name="eop", bufs=nj + 2))
        z_pool = ctx.enter_context(tc.tile_pool(name="zp", bufs=2))
        e_pool = ctx.enter_context(tc.tile_pool(name="ep", bufs=3))
        scores_pool = ctx.enter_context(tc.tile_pool(name="scoresp", bufs=2))
        attn_pool = ctx.enter_context(tc.tile_pool(name="attnp", bufs=2))
        small = ctx.enter_context(tc.tile_pool(name="small", bufs=2))
        ps_ct = ctx.enter_context(tc.tile_pool(name="psct", bufs=2, space="PSUM"))
        ps_dec = ctx.enter_context(tc.tile_pool(name="psdec", bufs=1, space="PSUM"))

        nk = nin // PT
        # ---- constants / small inputs (all on the GpSimd SWDGE queue)
        w_sb = singles.tile([PT, nk, h], F32)
        nc.gpsimd.dma_start(w_sb, wdec.rearrange("(c p) h -> p c h", p=PT))
        st_sb = singles.tile([PT, nk, bl], F32)
        nc.gpsimd.dma_start(st_sb, stT.rearrange("(c p) b -> p c b", p=PT))
        bdec_sb = singles.tile([1, h], F32)
        nc.gpsimd.dma_start(bdec_sb, bdec)
        ones1 = singles.tile([1, bl], F32)
        nc.vector.memset(ones1, 1.0)
        # v broadcast to all 128 partitions via stride-0 DMA from DRAM
        v_bc = singles.tile([PT, h], F32)
        nc.gpsimd.dma_start(
            v_bc, bass.AP(tensor=vw.tensor, offset=vw.offset, ap=[[0, PT], [1, h]])
        )

        # ---- dec_fea = s_t_hat @ W_dec + b_dec  (exact fp32) -> [bl, h]
        dec_ps = ps_dec.tile([bl, h], F32)
        for c in range(nk):
            nc.tensor.matmul(
                dec_ps, lhsT=st_sb[:, c, :], rhs=w_sb[:, c, :],
                start=(c == 0), stop=False,
            )
        nc.tensor.matmul(dec_ps, lhsT=ones1, rhs=bdec_sb, start=False, stop=True)
        dec_sb = singles.tile([bl, h], F32)
        nc.vector.tensor_copy(dec_sb, dec_ps)
        # broadcast each batch row to all partitions via DRAM round-trip
        dec_dram = dram_pool.tile([bl, h], F32)
        nc.gpsimd.dma_start(dec_dram, dec_sb)
        dec_bc = singles.tile([PT, bl, h], F32)
        for b in range(bl):
            nc.gpsimd.dma_start(
                dec_bc[:, b, :],
                bass.AP(tensor=dec_dram.tensor, offset=dec_dram.offset + b * h,
                        ap=[[0, PT], [1, h]]),
            )

        # ---- main loop
        for b in range(bl):
            scores_t = scores_pool.tile([PT, ncols], F32)
            for j in range(nj):
                ef_t = ef_pool.tile([PT, KT, h], F32)
                nc.sync.dma_start(ef_t, ef_v[b, j])
                z_t = z_pool.tile([PT, KT * h], F32)
                dslice = dec_bc[:, b, :]
                dec_rep = bass.AP(
                    tensor=dslice.tensor, offset=dslice.offset,
                    ap=[list(dslice.ap[0]), [0, KT], list(dslice.ap[1])],
                )
                nc.vector.tensor_add(
                    z_t.rearrange("p (k h) -> p k h", k=KT),
                    ef_t, dec_rep,
                )
                e_t = e_pool.tile([PT, KT, h], F32)
                nc.scalar.activation(
                    e_t.rearrange("p k h -> p (k h)"), z_t,
                    mybir.ActivationFunctionType.Tanh,
                )
                for k in range(KT):
                    dummy = small.tile([PT, 1], F32)
                    nc.vector.affine_mul_reduce(
                        dummy.broadcast_to([PT, h]),
                        scores_t[:, j * KT + k : j * KT + k + 1],
                        e_t[:, k, :], v_bc, 1.0, 0.0,
                    )

            # ---- softmax over t. No max-subtraction: |scores| <= sum|v_w|
            # (~8), exp cannot overflow in fp32, and softmax is shift-exact.
            ex = attn_pool.tile([PT, ncols], F32)
            rowsum = small.tile([PT, 1], F32)
            nc.scalar.activation(
                ex, scores_t, mybir.ActivationFunctionType.Exp,
                accum_out=rowsum,
            )
            # unnormalized weights feed the PE right away; 1/Z is applied to
            # c_t afterwards, so the all-reduce is off the critical path
            ex_r = attn_pool.tile([PT, ncols], F32R)
            nc.vector.tensor_copy(ex_r, ex)
            zsum = small.tile([PT, 1], F32)
            nc.gpsimd.partition_all_reduce(zsum, rowsum, PT, bass_isa.ReduceOp.add)
            rz = small.tile([PT, 1], F32)
            nc.vector.reciprocal(rz, zsum)
            at = attn_pool.tile([PT, ncols], F32)
            nc.vector.tensor_scalar_mul(at, ex, rz)

            # ---- attn output: strided scatter DMA (16KB per batch)
            nc.gpsimd.dma_start(attn_dram_ap(b), at)

            # ---- c_t = exp @ encoder_outputs * (1/Z)  (fp32r PE)
            # eo loads issued lazily here (Scalar HWDGE queue): issuing them
            # earlier steals HBM bandwidth from the latency-critical ef stream
            ct_ps = ps_ct.tile([1, h], F32)
            for j in range(nj):
                eo_t = eo_pool.tile([PT, KT, h], F32R)
                nc.scalar.dma_start(eo_t, eo_v[b, j].bitcast(F32R))
                for k in range(KT):
                    jk = j * KT + k
                    nc.tensor.matmul(
                        ct_ps, lhsT=ex_r[:, jk : jk + 1], rhs=eo_t[:, k, :],
                        start=(jk == 0), stop=(jk == nj * KT - 1),
                    )
            ct_sb = small.tile([1, h], F32)
            nc.vector.tensor_scalar_mul(ct_sb, ct_ps, rz[0:1, :])
            nc.gpsimd.dma_start(ct_out[b : b + 1, :], ct_sb)

    nc.compile()
    return nc


_NC_CACHE = {}


def _get_nc():
    if "nc" not in _NC_CACHE:
        _NC_CACHE["nc"] = build()
    return _NC_CACHE["nc"]


def make_in_maps(s_t_hat, encoder_outputs, encoder_feature, W_dec, b_dec, v_w,
                 bl=BL, ncores=NCORES):
    s_t_hat = np.asarray(s_t_hat, dtype=np.float32)
    encoder_outputs = np.asarray(encoder_outputs, dtype=np.float32)
    encoder_feature = np.asarray(encoder_feature, dtype=np.float32)
    W_dec = np.ascontiguousarray(np.asarray(W_dec, dtype=np.float32))
    b_dec = np.ascontiguousarray(np.asarray(b_dec, dtype=np.float32).reshape(1, -1))
    v_w = np.ascontiguousarray(np.asarray(v_w, dtype=np.float32).reshape(1, -1))
    b = s_t_hat.shape[0]
    t = encoder_outputs.shape[1]
    hh = encoder_outputs.shape[2]
    ef3 = encoder_feature.reshape(b, t, hh)
    in_maps = []
    for i in range(ncores):
        sl = slice(i * bl, (i + 1) * bl)
        in_maps.append({
            "ef": np.ascontiguousarray(ef3[sl].reshape(bl * t, hh)),
            "eo": np.ascontiguousarray(encoder_outputs[sl]),
            "stT": np.ascontiguousarray(s_t_hat[sl].T),
            "wdec": W_dec,
            "bdec": b_dec,
            "vw": v_w,
        })
    return in_maps


def kernel(s_t_hat, encoder_outputs, encoder_feature, coverage, W_dec, b_dec, v_w):
    coverage = np.asarray(coverage, dtype=np.float32)
    nc = _get_nc()
    in_maps = make_in_maps(
        s_t_hat, encoder_outputs, encoder_feature, W_dec, b_dec, v_w
    )
    res = None
    last_err = None
    for attempt in range(3):
        try:
            res = run_bass_kernel_spmd(nc, in_maps, list(range(NCORES))).results
            break
        except Exception as ex:  # transient NRT_EXEC_UNIT_UNRECOVERABLE flake
            last_err = ex
            try:
                import jax
                jax.clear_caches()
                jax.extend.backend.clear_backends()
            except Exception:
                pass
    if res is None:
        raise last_err
    c_t = np.concatenate([res[i]["ct"] for i in range(NCORES)], axis=0)
    attn = np.concatenate([res[i]["attn"] for i in range(NCORES)], axis=0)
    return (c_t, attn, coverage)
